# revision 19
# baseline (speedup 1.0000x reference)
"""Trainium2 Bass kernel for a 2-layer GCN (gnn_message_passing).

Math: out = Ahat @ (relu(Ahat @ (z W1) + b1) W2) + b2, with
Ahat = D^-1/2 (A+I) D^-1/2. The norm factorizes per-edge:
norm(src,dst) = dinv[src]*dinv[dst], so we pre-scale node features by
dinv, aggregate unweighted (pure segment-sum over edges), and
post-scale each destination row by dinv[dst].

Distribution (8 cores): nodes are permuted into 8*49 blocks of 128;
each core owns 49 destination blocks (bin-packed by in-edge count) and
all edges pointing into them. Per layer: every core computes the scaled
GEMM for its own node shard, an AllGather replicates the full scaled
feature table, then each core dma_gather's the source rows of its
blocks' in-edges and reduces them with one-hot matmuls accumulated in
PSUM (segment-sum by destination). Host does index preprocessing only
(degree counts, sort/bucket by dst block, int16 gather indices).
"""

import math
import sys

import numpy as np

sys.path.insert(0, "/opt/trn_rl_repo")

P = 128
NC_DEFAULT = 8
HALF = 32768  # int16 gather-index limit splits the node table in two


def _ceil_div(a, b):
    return (a + b - 1) // b


class GCNPlan:
    """Host-side preprocessing: node permutation, per-core edge buckets,
    padded chunk layout shared by all cores (SPMD)."""

    def __init__(self, z, ei, W1, b1, W2, b2, n_cores=NC_DEFAULT):
        N, Fin = z.shape
        F = W1.shape[1]
        E = ei.shape[1]
        self.N, self.Fin, self.F, self.NC = N, Fin, F, n_cores

        src = np.concatenate([np.asarray(ei[0]), np.arange(N, dtype=np.int64)])
        dst = np.concatenate([np.asarray(ei[1]), np.arange(N, dtype=np.int64)])
        deg = np.bincount(dst, minlength=N)
        dinv = np.zeros(N, np.float32)
        nz = deg > 0
        dinv[nz] = 1.0 / np.sqrt(deg[nz].astype(np.float64))

        BPC = _ceil_div(N, P * n_cores)  # blocks per core
        NBLK = BPC * n_cores
        N_pad = NBLK * P
        self.BPC, self.NBLK, self.N_pad = BPC, NBLK, N_pad
        self.CP = BPC * P  # nodes per core

        # ---- assign dst blocks to cores, balanced by edge count ----
        gblk = (dst // P).astype(np.int64)
        blk_cnt = np.bincount(gblk, minlength=NBLK)
        order = np.argsort(-blk_cnt, kind="stable")
        core_tot = np.zeros(n_cores, np.int64)
        core_blocks = [[] for _ in range(n_cores)]
        for g in order:
            avail = [c for c in range(n_cores) if len(core_blocks[c]) < BPC]
            c = min(avail, key=lambda c: core_tot[c])
            core_blocks[c].append(int(g))
            core_tot[c] += blk_cnt[g]
        self.core_blocks = core_blocks  # [c][i] = global block id

        # node permutation: block g -> (core c, position i) -> table rows
        perm = np.empty(N_pad, np.int64)
        for c in range(n_cores):
            for i, g in enumerate(core_blocks[c]):
                base = c * self.CP + i * P
                perm[g * P:(g + 1) * P] = base + np.arange(P)
        self.perm = perm

        # ---- bucket edges by destination block ----
        # Feature tables are stored as [N_pad/2, 2F] (a pair of nodes per
        # row): gather index = src//2 fits int16, descriptors are 512B
        # (bf16) and half as many; chunks are grouped by src parity so the
        # matmul reads the correct half of each gathered row.
        eorder = np.argsort(gblk, kind="stable")
        g_sorted = gblk[eorder]
        bs = np.searchsorted(g_sorted, np.arange(NBLK + 1))
        sp_all = perm[src[eorder]]  # permuted source index
        dl_all = (dst % P)[eorder].astype(np.int64)

        # per (core, position): even/odd-parity lists
        ev_idx = [[None] * BPC for _ in range(n_cores)]
        od_idx = [[None] * BPC for _ in range(n_cores)]
        ev_dl = [[None] * BPC for _ in range(n_cores)]
        od_dl = [[None] * BPC for _ in range(n_cores)]
        for c in range(n_cores):
            for i, g in enumerate(core_blocks[c]):
                s, e = bs[g], bs[g + 1]
                sp = sp_all[s:e]
                dl = dl_all[s:e]
                m = (sp % 2) == 0
                ev_idx[c][i] = (sp[m] // 2).astype(np.int64)
                ev_dl[c][i] = dl[m]
                od_idx[c][i] = (sp[~m] // 2).astype(np.int64)
                od_dl[c][i] = dl[~m]

        # shared (SPMD) chunk counts per position: max over cores
        self.K_EV = [
            max(_ceil_div(len(ev_idx[c][i]), P) for c in range(n_cores))
            for i in range(BPC)
        ]
        self.K_OD = [
            max(_ceil_div(len(od_idx[c][i]), P) for c in range(n_cores))
            for i in range(BPC)
        ]
        # ensure at least one chunk per position so PSUM is always written
        for i in range(BPC):
            if self.K_EV[i] == 0 and self.K_OD[i] == 0:
                self.K_EV[i] = 1
        self.TCH = sum(self.K_EV[i] + self.K_OD[i] for i in range(BPC))
        self.TIDX = self.TCH * 8  # int16 columns (16 idx per column group)

        # ---- build per-core device arrays ----
        padded_edges = self.TCH * P * n_cores
        real_edges = E + N
        self.pad_frac = padded_edges / real_edges - 1.0

        self.gidx = np.zeros((n_cores, P, self.TIDX), np.int16)
        self.dloc = np.full((n_cores, P, self.TCH), 200.0, np.float32)
        for c in range(n_cores):
            col = 0  # chunk cursor
            for i in range(BPC):
                for idxs, dls, Kq in (
                    (ev_idx[c][i], ev_dl[c][i], self.K_EV[i]),
                    (od_idx[c][i], od_dl[c][i], self.K_OD[i]),
                ):
                    L = Kq * P
                    if L == 0:
                        continue
                    buf = np.zeros(L, np.int64)
                    buf[: len(idxs)] = idxs
                    dbuf = np.full(L, 200.0, np.float32)
                    dbuf[: len(dls)] = dls
                    wrapped = buf.reshape(L // 16, 16).T.astype(np.int16)
                    self.gidx[c][:, col * 8: col * 8 + L // 16] = np.tile(
                        wrapped, (8, 1)
                    )
                    self.dloc[c][:, col: col + Kq] = dbuf.reshape(Kq, P).T
                    col += Kq
            assert col == self.TCH

        # ---- dense per-core inputs ----
        zp = np.zeros((N_pad, Fin), np.float32)
        zp[perm[:N]] = np.asarray(z, np.float32)
        dinvp = np.zeros(N_pad, np.float32)
        dinvp[perm[:N]] = dinv

        self.zT = np.ascontiguousarray(
            zp.reshape(n_cores, self.CP, Fin).transpose(0, 2, 1)
        )  # [c, Fin, CP]
        dpc = dinvp.reshape(n_cores, self.CP)
        self.dinvc = np.ascontiguousarray(
            dpc.reshape(n_cores, BPC, P).transpose(0, 2, 1)
        )  # [c, P, BPC]
        self.dinvb = np.ascontiguousarray(
            np.broadcast_to(dpc[:, None, :], (n_cores, P, self.CP))
        )  # [c, P, CP]

        self.W1 = np.asarray(W1, np.float32)
        self.W2 = np.asarray(W2, np.float32)
        self.b1c = np.asarray(b1, np.float32).reshape(F, 1)
        self.b2b = np.tile(np.asarray(b2, np.float32)[None, :], (P, 1))
        self.iota = np.tile(np.arange(P, dtype=np.float32)[None, :], (P, 1))


def build_bass(plan, table_dtype="float32", phases="ABCD", agg_mode="full",
               no_cc=False):
    """Build the SPMD Bass program (same for all cores)."""
    from concourse import bacc, mybir
    from concourse import bass
    from concourse import library_config
    import concourse.tile as tile
    from concourse.tile_rust import add_dep_helper

    dt = mybir.dt
    DT = getattr(dt, table_dtype)
    f32 = dt.float32
    NC = plan.NC
    BPC, CP, F, Fin = plan.BPC, plan.CP, plan.F, plan.Fin
    N_pad = plan.N_pad
    Relu = mybir.ActivationFunctionType.Relu
    is_eq = mybir.AluOpType.is_equal

    nc = bacc.Bacc(
        "TRN2", target_bir_lowering=False, debug=False, num_devices=NC
    )

    zT_d = nc.dram_tensor("zT", [Fin, CP], f32, kind="ExternalInput")
    W1_d = nc.dram_tensor("W1", [Fin, F], f32, kind="ExternalInput")
    W2_d = nc.dram_tensor("W2", [F, F], f32, kind="ExternalInput")
    b1_d = nc.dram_tensor("b1c", [F, 1], f32, kind="ExternalInput")
    b2_d = nc.dram_tensor("b2b", [P, F], f32, kind="ExternalInput")
    iota_d = nc.dram_tensor("iota", [P, P], DT, kind="ExternalInput")
    dinvc_d = nc.dram_tensor("dinvc", [P, BPC], f32, kind="ExternalInput")
    dinvb_d = nc.dram_tensor("dinvb", [P, CP], f32, kind="ExternalInput")
    gidx_d = nc.dram_tensor("gidx", [P, plan.TIDX], dt.int16, kind="ExternalInput")
    dloc_d = nc.dram_tensor("dloc", [P, plan.TCH], DT, kind="ExternalInput")
    out_d = nc.dram_tensor("out", [CP, F], f32, kind="ExternalOutput")

    # tables are stored pair-packed: row r holds nodes 2r and 2r+1
    x1loc = nc.dram_tensor("x1loc", [CP, F], DT)
    x1full = nc.dram_tensor("x1full", [N_pad // 2, 2 * F], DT, addr_space="Shared")
    x2loc = nc.dram_tensor("x2loc", [CP, F], DT)
    x2full = nc.dram_tensor("x2full", [N_pad // 2, 2 * F], DT, addr_space="Shared")

    rg = [list(range(NC))]

    with tile.TileContext(nc) as tc:
        with (
            tc.tile_pool(name="const", bufs=1) as cpool,
            tc.tile_pool(name="gather", bufs=3) as gpool,
            tc.tile_pool(name="oh", bufs=6) as ohpool,
            tc.tile_pool(name="ep", bufs=4) as eppool,
            tc.tile_pool(name="psum", bufs=6, space="PSUM") as pspool,
        ):
            lib = nc.gpsimd.load_library(library_config.mlp)

            zT_sb = cpool.tile([Fin, CP], f32)
            nc.sync.dma_start(zT_sb[:], zT_d[:])
            W1_sb = cpool.tile([Fin, F], f32)
            nc.sync.dma_start(W1_sb[:], W1_d[:])
            W2_sb = cpool.tile([F, F], f32)
            nc.sync.dma_start(W2_sb[:], W2_d[:])
            b1_sb = cpool.tile([F, 1], f32)
            nc.sync.dma_start(b1_sb[:], b1_d[:])
            b2_sb = cpool.tile([P, F], f32)
            nc.sync.dma_start(b2_sb[:], b2_d[:])
            iota_sb = cpool.tile([P, P], DT)
            nc.sync.dma_start(iota_sb[:], iota_d[:])
            dinvc_sb = cpool.tile([P, BPC], f32)
            nc.sync.dma_start(dinvc_sb[:], dinvc_d[:])
            dinvb_sb = cpool.tile([P, CP], f32)
            nc.sync.dma_start(dinvb_sb[:], dinvb_d[:])
            gidx_sb = cpool.tile([P, plan.TIDX], dt.int16)
            nc.sync.dma_start(gidx_sb[:], gidx_d[:])
            dloc_sb = cpool.tile([P, plan.TCH], DT)
            nc.sync.dma_start(dloc_sb[:], dloc_d[:])
            hT_sb = cpool.tile([F, CP], f32)

            # ---- phase A: x1 = dinv * (z @ W1), local shard ----
            for i in range(BPC):
                ps = pspool.tile([P, F], f32, space="PSUM", tag="ps")
                nc.tensor.matmul(
                    ps[:], lhsT=zT_sb[:, i * P:(i + 1) * P], rhs=W1_sb[:],
                    start=True, stop=True,
                )
                x1t = eppool.tile([P, F], DT, tag="xo")
                nc.vector.tensor_scalar_mul(x1t[:], ps[:], dinvc_sb[:, i:i + 1])
                nc.sync.dma_start(x1loc[i * P:(i + 1) * P, :], x1t[:])

            if no_cc:
                nc.sync.dma_start(x1full[0:CP // 2, :], x1loc[:])
            else:
                nc.gpsimd.collective_compute(
                    "AllGather", mybir.AluOpType.bypass,
                    ins=[x1loc[:]], outs=[x1full[:]], replica_groups=rg,
                )

            def aggregate(table, variant, consume):
                """Per dst block: gather src pair-rows, one-hot matmul
                segment sum. variant 'T': psum[feat, dst]; 'N': psum[dst,
                feat]."""
                col = 0
                for i in range(BPC):
                    kev, kod = plan.K_EV[i], plan.K_OD[i]
                    K = kev + kod
                    Xg = gpool.tile([P, K, 2 * F], DT, tag="Xg")
                    if agg_mode == "no_gather":
                        for k0 in range(K):
                            nc.sync.dma_start(
                                Xg[:, k0, :], table[k0 * P:(k0 + 1) * P, :]
                            )
                    else:
                        g = nc.gpsimd.dma_gather(
                            Xg[:, :, :],
                            table[:, :],
                            gidx_sb[:, col * 8:(col + K) * 8],
                            K * P,
                            K * P,
                            2 * F,
                            # single-packet mode caps at 64 descriptors per
                            # SDMA engine (1024 idxs) — beyond that the
                            # device wedges
                            single_packet=(K * P <= 1024),
                        )
                        add_dep_helper(lib.ins, g.ins, sync=True,
                                       reason="lib before gather")
                    if agg_mode == "gather_only":
                        ps = pspool.tile([P, P], f32, space="PSUM", tag="ps")
                        nc.tensor.matmul(ps[:], lhsT=Xg[:, 0, 0:F],
                                         rhs=iota_sb[:], start=True, stop=True)
                        consume(i, ps)
                        col += K
                        continue
                    ps = pspool.tile([P, P], f32, space="PSUM", tag="ps")
                    for cch in range(K):
                        par = 0 if cch < kev else 1
                        oh = ohpool.tile([P, P], DT, tag="oh")
                        nc.vector.tensor_tensor(
                            out=oh[:],
                            in0=dloc_sb[:, col + cch:col + cch + 1].to_broadcast(
                                [P, P]
                            ),
                            in1=iota_sb[:],
                            op=is_eq,
                        )
                        xsl = Xg[:, cch, par * F:(par + 1) * F]
                        if variant == "T":
                            nc.tensor.matmul(
                                ps[:], lhsT=xsl, rhs=oh[:],
                                start=(cch == 0), stop=(cch == K - 1),
                            )
                        else:
                            nc.tensor.matmul(
                                ps[:], lhsT=oh[:], rhs=xsl,
                                start=(cch == 0), stop=(cch == K - 1),
                            )
                    consume(i, ps)
                    col += K

            # ---- phase B: layer-1 aggregation -> hT (SBUF resident) ----
            def consume1(i, ps):
                tmp = eppool.tile([P, P], f32, tag="tmp")
                nc.vector.tensor_mul(
                    tmp[:], ps[:], dinvb_sb[:, i * P:(i + 1) * P]
                )
                nc.scalar.activation(
                    hT_sb[:, i * P:(i + 1) * P], tmp[:], Relu, bias=b1_sb[:, 0:1]
                )

            if "B" in phases:
                aggregate(x1full, "T", consume1)
            else:
                for i in range(BPC):
                    z0 = eppool.tile([P, P], f32, tag="tmp")
                    nc.vector.tensor_scalar_mul(z0[:], iota_sb[:], 0.0)
                    nc.scalar.activation(
                        hT_sb[:, i * P:(i + 1) * P], z0[:], Relu,
                        bias=b1_sb[:, 0:1],
                    )

            if "C" not in phases:
                for i in range(BPC):
                    o = eppool.tile([P, F], f32, tag="xo")
                    nc.vector.tensor_copy(o[:], hT_sb[:, i * P:(i + 1) * P])
                    nc.sync.dma_start(out_d[i * P:(i + 1) * P, :], o[:])
            else:
                # ---- phase C: x2 = dinv * (h @ W2), local shard ----
                for i in range(BPC):
                    ps = pspool.tile([P, F], f32, space="PSUM", tag="ps")
                    nc.tensor.matmul(
                        ps[:], lhsT=hT_sb[:, i * P:(i + 1) * P], rhs=W2_sb[:],
                        start=True, stop=True,
                    )
                    x2t = eppool.tile([P, F], DT, tag="xo")
                    nc.vector.tensor_scalar_mul(
                        x2t[:], ps[:], dinvc_sb[:, i:i + 1]
                    )
                    nc.sync.dma_start(x2loc[i * P:(i + 1) * P, :], x2t[:])

                if no_cc:
                    nc.sync.dma_start(x2full[0:CP // 2, :], x2loc[:])
                else:
                    nc.gpsimd.collective_compute(
                        "AllGather", mybir.AluOpType.bypass,
                        ins=[x2loc[:]], outs=[x2full[:]], replica_groups=rg,
                    )

            # ---- phase D: layer-2 aggregation -> out rows ----
            def consume2(i, ps):
                t1 = eppool.tile([P, F], f32, tag="tmp")
                nc.vector.tensor_scalar_mul(t1[:], ps[:], dinvc_sb[:, i:i + 1])
                o = eppool.tile([P, F], f32, tag="xo")
                nc.vector.tensor_add(o[:], t1[:], b2_sb[:])
                nc.sync.dma_start(out_d[i * P:(i + 1) * P, :], o[:])

            if "C" in phases:
                if "D" in phases:
                    aggregate(x2full, "N", consume2)
                else:
                    for i in range(BPC):
                        o = eppool.tile([P, F], f32, tag="xo")
                        nc.sync.dma_start(o[:], x2full[i * P:(i + 1) * P, :])
                        nc.sync.dma_start(out_d[i * P:(i + 1) * P, :], o[:])

    nc.compile()
    return nc


def make_in_maps(plan, table_dtype="float32"):
    import ml_dtypes

    cast = {
        "float32": np.float32,
        "bfloat16": ml_dtypes.bfloat16,
    }[table_dtype]
    maps = []
    for c in range(plan.NC):
        maps.append(
            {
                "zT": plan.zT[c],
                "W1": plan.W1,
                "W2": plan.W2,
                "b1c": plan.b1c,
                "b2b": plan.b2b,
                "iota": plan.iota.astype(cast),
                "dinvc": plan.dinvc[c],
                "dinvb": plan.dinvb[c],
                "gidx": plan.gidx[c],
                "dloc": plan.dloc[c].astype(cast),
            }
        )
    return maps


_CACHE = {}


def _run(z, ei, W1, b1, W2, b2, n_cores=NC_DEFAULT, table_dtype="float32",
         trace=False):
    from concourse.bass_utils import run_bass_kernel_spmd

    plan = GCNPlan(z, ei, W1, b1, W2, b2, n_cores=n_cores)
    key = (plan.N, plan.TCH, table_dtype)
    if key not in _CACHE:
        _CACHE[key] = build_bass(plan, table_dtype=table_dtype)
    nc = _CACHE[key]
    in_maps = make_in_maps(plan, table_dtype=table_dtype)
    res = run_bass_kernel_spmd(
        nc, in_maps, list(range(n_cores)), trace=trace
    )
    outp = np.concatenate([res.results[c]["out"] for c in range(plan.NC)], axis=0)
    out = outp[plan.perm[: plan.N]].astype(np.float32)
    return out, res


def kernel(z, ei, W1, b1, W2, b2):
    out, _ = _run(
        np.asarray(z), np.asarray(ei), np.asarray(W1), np.asarray(b1),
        np.asarray(W2), np.asarray(b2), table_dtype="float32",
    )
    return out


# revision 30
# speedup vs baseline: 4.5357x; 4.5357x over previous
"""Trainium2 Bass kernel for a 2-layer GCN (gnn_message_passing).

Math: out = Ahat @ (relu(Ahat @ (z W1) + b1) W2) + b2, with
Ahat = D^-1/2 (A+I) D^-1/2. The norm factorizes per-edge:
norm(src,dst) = dinv[src]*dinv[dst], so we pre-scale node features by
dinv, aggregate unweighted (pure segment-sum over edges), and
post-scale each destination row by dinv[dst].

Distribution (8 cores): nodes are permuted into 8*49 blocks of 128;
each core owns 49 destination blocks (bin-packed by in-edge count) and
all edges pointing into them. Per layer: every core computes the scaled
GEMM for its own node shard, an AllGather replicates the full scaled
feature table, then each core dma_gather's the source rows of its
blocks' in-edges and reduces them with one-hot matmuls accumulated in
PSUM (segment-sum by destination). Host does index preprocessing only
(degree counts, sort/bucket by dst block, int16 gather indices).
"""

import math
import sys

import numpy as np

sys.path.insert(0, "/opt/trn_rl_repo")

P = 128
NC_DEFAULT = 8
HALF = 32768  # int16 gather-index limit splits the node table in two


def _ceil_div(a, b):
    return (a + b - 1) // b


class GCNPlan:
    """Host-side preprocessing: node permutation, per-core edge buckets,
    padded chunk layout shared by all cores (SPMD)."""

    def __init__(self, z, ei, W1, b1, W2, b2, n_cores=NC_DEFAULT):
        N, Fin = z.shape
        F = W1.shape[1]
        E = ei.shape[1]
        self.N, self.Fin, self.F, self.NC = N, Fin, F, n_cores

        src = np.concatenate([np.asarray(ei[0]), np.arange(N, dtype=np.int64)])
        dst = np.concatenate([np.asarray(ei[1]), np.arange(N, dtype=np.int64)])
        deg = np.bincount(dst, minlength=N)
        dinv = np.zeros(N, np.float32)
        nz = deg > 0
        dinv[nz] = 1.0 / np.sqrt(deg[nz].astype(np.float64))

        BPC = _ceil_div(N, P * n_cores)  # blocks per core
        NBLK = BPC * n_cores
        N_pad = NBLK * P
        self.BPC, self.NBLK, self.N_pad = BPC, NBLK, N_pad
        self.CP = BPC * P  # nodes per core

        # ---- assign dst blocks to cores, balanced by edge count ----
        gblk = (dst // P).astype(np.int64)
        blk_cnt = np.bincount(gblk, minlength=NBLK)
        order = np.argsort(-blk_cnt, kind="stable")
        core_tot = np.zeros(n_cores, np.int64)
        core_blocks = [[] for _ in range(n_cores)]
        for g in order:
            avail = [c for c in range(n_cores) if len(core_blocks[c]) < BPC]
            c = min(avail, key=lambda c: core_tot[c])
            core_blocks[c].append(int(g))
            core_tot[c] += blk_cnt[g]
        self.core_blocks = core_blocks  # [c][i] = global block id

        # node permutation: block g -> (core c, position i) -> table rows
        perm = np.empty(N_pad, np.int64)
        for c in range(n_cores):
            for i, g in enumerate(core_blocks[c]):
                base = c * self.CP + i * P
                perm[g * P:(g + 1) * P] = base + np.arange(P)
        self.perm = perm

        # ---- bucket edges by destination block ----
        # Feature tables are stored as [N_pad/2, 2F] (a pair of nodes per
        # row): gather index = src//2 fits int16, descriptors are 512B
        # (bf16) and half as many; chunks are grouped by src parity so the
        # matmul reads the correct half of each gathered row.
        eorder = np.argsort(gblk, kind="stable")
        g_sorted = gblk[eorder]
        bs = np.searchsorted(g_sorted, np.arange(NBLK + 1))
        sp_all = perm[src[eorder]]  # permuted source index
        dl_all = (dst % P)[eorder].astype(np.int64)

        # per (core, position): even/odd-parity lists
        ev_idx = [[None] * BPC for _ in range(n_cores)]
        od_idx = [[None] * BPC for _ in range(n_cores)]
        ev_dl = [[None] * BPC for _ in range(n_cores)]
        od_dl = [[None] * BPC for _ in range(n_cores)]
        for c in range(n_cores):
            for i, g in enumerate(core_blocks[c]):
                s, e = bs[g], bs[g + 1]
                sp = sp_all[s:e]
                dl = dl_all[s:e]
                m = (sp % 2) == 0
                ev_idx[c][i] = (sp[m] // 2).astype(np.int64)
                ev_dl[c][i] = dl[m]
                od_idx[c][i] = (sp[~m] // 2).astype(np.int64)
                od_dl[c][i] = dl[~m]

        # shared (SPMD) chunk counts per position: max over cores
        self.K_EV = [
            max(_ceil_div(len(ev_idx[c][i]), P) for c in range(n_cores))
            for i in range(BPC)
        ]
        self.K_OD = [
            max(_ceil_div(len(od_idx[c][i]), P) for c in range(n_cores))
            for i in range(BPC)
        ]
        # ensure at least one chunk per position so PSUM is always written
        for i in range(BPC):
            if self.K_EV[i] == 0 and self.K_OD[i] == 0:
                self.K_EV[i] = 1
        self.TCH = sum(self.K_EV[i] + self.K_OD[i] for i in range(BPC))
        self.TIDX = self.TCH * 8  # int16 columns (16 idx per column group)

        # ---- build per-core device arrays ----
        padded_edges = self.TCH * P * n_cores
        real_edges = E + N
        self.pad_frac = padded_edges / real_edges - 1.0

        self.gidx = np.zeros((n_cores, P, self.TIDX), np.int16)
        self.dloc = np.full((n_cores, P, self.TCH), 200.0, np.float32)
        for c in range(n_cores):
            col = 0  # chunk cursor
            for i in range(BPC):
                for idxs, dls, Kq in (
                    (ev_idx[c][i], ev_dl[c][i], self.K_EV[i]),
                    (od_idx[c][i], od_dl[c][i], self.K_OD[i]),
                ):
                    L = Kq * P
                    if L == 0:
                        continue
                    buf = np.zeros(L, np.int64)
                    buf[: len(idxs)] = idxs
                    dbuf = np.full(L, 200.0, np.float32)
                    dbuf[: len(dls)] = dls
                    wrapped = buf.reshape(L // 16, 16).T.astype(np.int16)
                    self.gidx[c][:, col * 8: col * 8 + L // 16] = np.tile(
                        wrapped, (8, 1)
                    )
                    self.dloc[c][:, col: col + Kq] = dbuf.reshape(Kq, P).T
                    col += Kq
            assert col == self.TCH

        # ---- dense per-core inputs ----
        zp = np.zeros((N_pad, Fin), np.float32)
        zp[perm[:N]] = np.asarray(z, np.float32)
        dinvp = np.zeros(N_pad, np.float32)
        dinvp[perm[:N]] = dinv

        self.zT = np.ascontiguousarray(
            zp.reshape(n_cores, self.CP, Fin).transpose(0, 2, 1)
        )  # [c, Fin, CP]
        dpc = dinvp.reshape(n_cores, self.CP)
        self.dinvc = np.ascontiguousarray(
            dpc.reshape(n_cores, BPC, P).transpose(0, 2, 1)
        )  # [c, P, BPC]
        self.dinvb = np.ascontiguousarray(
            np.broadcast_to(dpc[:, None, :], (n_cores, P, self.CP))
        )  # [c, P, CP]

        self.W1 = np.asarray(W1, np.float32)
        self.W2 = np.asarray(W2, np.float32)
        self.b1c = np.asarray(b1, np.float32).reshape(F, 1)
        self.b2b = np.tile(np.asarray(b2, np.float32)[None, :], (P, 1))
        self.iota = np.tile(np.arange(P, dtype=np.float32)[None, :], (P, 1))
        self.KMAX = max(
            self.K_EV[i] + self.K_OD[i] for i in range(BPC)
        )
        self.iotar = np.tile(
            np.arange(P, dtype=np.float32)[None, None, :], (P, self.KMAX, 1)
        )  # [P, KMAX, P]


def build_bass(plan, table_dtype="float32", phases="ABCD", agg_mode="full",
               no_cc=False, repeat_b=1, n_queues=1, gather_split=0):
    """Build the SPMD Bass program (same for all cores)."""
    from concourse import bacc, mybir
    from concourse import bass
    from concourse import library_config
    import concourse.tile as tile
    from concourse.tile_rust import add_dep_helper

    dt = mybir.dt
    DT = getattr(dt, table_dtype)
    f32 = dt.float32
    NC = plan.NC
    BPC, CP, F, Fin = plan.BPC, plan.CP, plan.F, plan.Fin
    N_pad = plan.N_pad
    Relu = mybir.ActivationFunctionType.Relu
    is_eq = mybir.AluOpType.is_equal

    nc = bacc.Bacc(
        "TRN2", target_bir_lowering=False, debug=False, num_devices=NC,
        num_swdge_queues=max(1, n_queues),
    )
    queue_rr = [0]

    zT_d = nc.dram_tensor("zT", [Fin, CP], f32, kind="ExternalInput")
    W1_d = nc.dram_tensor("W1", [Fin, F], f32, kind="ExternalInput")
    W2_d = nc.dram_tensor("W2", [F, F], f32, kind="ExternalInput")
    b1_d = nc.dram_tensor("b1c", [F, 1], f32, kind="ExternalInput")
    b2_d = nc.dram_tensor("b2b", [P, F], f32, kind="ExternalInput")
    iota_d = nc.dram_tensor("iota", [P, P], DT, kind="ExternalInput")
    iotar_d = nc.dram_tensor("iotar", [P, plan.KMAX, P], DT, kind="ExternalInput")
    dinvc_d = nc.dram_tensor("dinvc", [P, BPC], f32, kind="ExternalInput")
    dinvb_d = nc.dram_tensor("dinvb", [P, CP], f32, kind="ExternalInput")
    gidx_d = nc.dram_tensor("gidx", [P, plan.TIDX], dt.int16, kind="ExternalInput")
    dloc_d = nc.dram_tensor("dloc", [P, plan.TCH], DT, kind="ExternalInput")
    out_d = nc.dram_tensor("out", [CP, F], f32, kind="ExternalOutput")

    # tables are stored pair-packed: row r holds nodes 2r and 2r+1
    x1loc = nc.dram_tensor("x1loc", [CP, F], DT)
    x1full = nc.dram_tensor("x1full", [N_pad // 2, 2 * F], DT, addr_space="Shared")
    x2loc = nc.dram_tensor("x2loc", [CP, F], DT)
    x2full = nc.dram_tensor("x2full", [N_pad // 2, 2 * F], DT, addr_space="Shared")

    rg = [list(range(NC))]

    with tile.TileContext(nc) as tc:
        with (
            tc.tile_pool(name="const", bufs=1) as cpool,
            tc.tile_pool(name="gather", bufs=3) as gpool,
            tc.tile_pool(name="oh", bufs=3) as ohpool,
            tc.tile_pool(name="ep", bufs=4) as eppool,
            tc.tile_pool(name="psum", bufs=6, space="PSUM") as pspool,
        ):
            lib = nc.gpsimd.load_library(library_config.mlp)

            zT_sb = cpool.tile([Fin, CP], f32)
            nc.sync.dma_start(zT_sb[:], zT_d[:])
            W1_sb = cpool.tile([Fin, F], f32)
            nc.sync.dma_start(W1_sb[:], W1_d[:])
            W2_sb = cpool.tile([F, F], f32)
            nc.sync.dma_start(W2_sb[:], W2_d[:])
            b1_sb = cpool.tile([F, 1], f32)
            nc.sync.dma_start(b1_sb[:], b1_d[:])
            b2_sb = cpool.tile([P, F], f32)
            nc.sync.dma_start(b2_sb[:], b2_d[:])
            iota_sb = cpool.tile([P, P], DT)
            nc.sync.dma_start(iota_sb[:], iota_d[:])
            iotar_sb = cpool.tile([P, plan.KMAX, P], DT)
            nc.sync.dma_start(iotar_sb[:], iotar_d[:])
            dinvc_sb = cpool.tile([P, BPC], f32)
            nc.sync.dma_start(dinvc_sb[:], dinvc_d[:])
            dinvb_sb = cpool.tile([P, CP], f32)
            nc.sync.dma_start(dinvb_sb[:], dinvb_d[:])
            gidx_sb = cpool.tile([P, plan.TIDX], dt.int16)
            nc.sync.dma_start(gidx_sb[:], gidx_d[:])
            dloc_sb = cpool.tile([P, plan.TCH], DT)
            nc.sync.dma_start(dloc_sb[:], dloc_d[:])
            hT_sb = cpool.tile([F, CP], f32)

            # ---- phase A: x1 = dinv * (z @ W1), local shard ----
            for i in range(BPC):
                ps = pspool.tile([P, F], f32, space="PSUM", tag="ps")
                nc.tensor.matmul(
                    ps[:], lhsT=zT_sb[:, i * P:(i + 1) * P], rhs=W1_sb[:],
                    start=True, stop=True,
                )
                x1t = eppool.tile([P, F], DT, tag="xo")
                nc.vector.tensor_scalar_mul(x1t[:], ps[:], dinvc_sb[:, i:i + 1])
                nc.sync.dma_start(x1loc[i * P:(i + 1) * P, :], x1t[:])

            if no_cc:
                nc.sync.dma_start(x1full[0:CP // 2, :], x1loc[:])
            else:
                nc.gpsimd.collective_compute(
                    "AllGather", mybir.AluOpType.bypass,
                    ins=[x1loc[:]], outs=[x1full[:]], replica_groups=rg,
                )

            def aggregate(table, variant, consume):
                """Per dst block: gather src pair-rows, one-hot matmul
                segment sum. variant 'T': psum[feat, dst]; 'N': psum[dst,
                feat]."""
                col = 0
                for i in range(BPC):
                    kev, kod = plan.K_EV[i], plan.K_OD[i]
                    K = kev + kod
                    Xg = gpool.tile([P, K, 2 * F], DT, tag="Xg")
                    if agg_mode == "no_gather":
                        for k0 in range(K):
                            nc.sync.dma_start(
                                Xg[:, k0, :], table[k0 * P:(k0 + 1) * P, :]
                            )
                    else:
                        if gather_split:
                            kstep = gather_split // P
                            pieces = [
                                (k0, min(kstep, K - k0))
                                for k0 in range(0, K, kstep)
                            ]
                        else:
                            pieces = [(0, K)]
                        for (k0, kq) in pieces:
                            g = nc.gpsimd.dma_gather(
                                Xg[:, k0:k0 + kq, :],
                                table[:, :],
                                gidx_sb[:, (col + k0) * 8:(col + k0 + kq) * 8],
                                kq * P,
                                kq * P,
                                2 * F,
                                # single-packet mode caps at 64 descriptors
                                # per SDMA engine (1024 idxs) — beyond that
                                # the device wedges
                                single_packet=(kq * P <= 1024),
                                queue_num=queue_rr[0] % max(1, n_queues),
                            )
                            queue_rr[0] += 1
                            add_dep_helper(lib.ins, g.ins, sync=True,
                                           reason="lib before gather")
                    if agg_mode == "gather_only":
                        ps = pspool.tile([P, P], f32, space="PSUM", tag="ps")
                        nc.tensor.matmul(ps[:], lhsT=Xg[:, 0, 0:F],
                                         rhs=iota_sb[:], start=True, stop=True)
                        consume(i, ps)
                        col += K
                        continue
                    ps = pspool.tile([P, P], f32, space="PSUM", tag="ps")
                    oh = ohpool.tile([P, K, P], DT, tag="oh")
                    nc.vector.tensor_tensor(
                        out=oh[:, :, :],
                        in0=dloc_sb[:, col:col + K].to_broadcast([P, K, P]),
                        in1=iotar_sb[:, 0:K, :],
                        op=is_eq,
                    )
                    for cch in range(K):
                        par = 0 if cch < kev else 1
                        xsl = Xg[:, cch, par * F:(par + 1) * F]
                        if variant == "T":
                            nc.tensor.matmul(
                                ps[:], lhsT=xsl, rhs=oh[:, cch, :],
                                start=(cch == 0), stop=(cch == K - 1),
                            )
                        else:
                            nc.tensor.matmul(
                                ps[:], lhsT=oh[:, cch, :], rhs=xsl,
                                start=(cch == 0), stop=(cch == K - 1),
                            )
                    consume(i, ps)
                    col += K

            # ---- phase B: layer-1 aggregation -> hT (SBUF resident) ----
            def consume1(i, ps):
                tmp = eppool.tile([P, P], f32, tag="tmp")
                nc.vector.tensor_mul(
                    tmp[:], ps[:], dinvb_sb[:, i * P:(i + 1) * P]
                )
                nc.scalar.activation(
                    hT_sb[:, i * P:(i + 1) * P], tmp[:], Relu, bias=b1_sb[:, 0:1]
                )

            if "B" in phases:
                for _rep in range(repeat_b):
                    aggregate(x1full, "T", consume1)
            else:
                for i in range(BPC):
                    z0 = eppool.tile([P, P], f32, tag="tmp")
                    nc.vector.tensor_scalar_mul(z0[:], iota_sb[:], 0.0)
                    nc.scalar.activation(
                        hT_sb[:, i * P:(i + 1) * P], z0[:], Relu,
                        bias=b1_sb[:, 0:1],
                    )

            if "C" not in phases:
                for i in range(BPC):
                    o = eppool.tile([P, F], f32, tag="xo")
                    nc.vector.tensor_copy(o[:], hT_sb[:, i * P:(i + 1) * P])
                    nc.sync.dma_start(out_d[i * P:(i + 1) * P, :], o[:])
            else:
                # ---- phase C: x2 = dinv * (h @ W2), local shard ----
                for i in range(BPC):
                    ps = pspool.tile([P, F], f32, space="PSUM", tag="ps")
                    nc.tensor.matmul(
                        ps[:], lhsT=hT_sb[:, i * P:(i + 1) * P], rhs=W2_sb[:],
                        start=True, stop=True,
                    )
                    x2t = eppool.tile([P, F], DT, tag="xo")
                    nc.vector.tensor_scalar_mul(
                        x2t[:], ps[:], dinvc_sb[:, i:i + 1]
                    )
                    nc.sync.dma_start(x2loc[i * P:(i + 1) * P, :], x2t[:])

                if no_cc:
                    nc.sync.dma_start(x2full[0:CP // 2, :], x2loc[:])
                else:
                    nc.gpsimd.collective_compute(
                        "AllGather", mybir.AluOpType.bypass,
                        ins=[x2loc[:]], outs=[x2full[:]], replica_groups=rg,
                    )

            # ---- phase D: layer-2 aggregation -> out rows ----
            def consume2(i, ps):
                t1 = eppool.tile([P, F], f32, tag="tmp")
                nc.vector.tensor_scalar_mul(t1[:], ps[:], dinvc_sb[:, i:i + 1])
                o = eppool.tile([P, F], f32, tag="xo")
                nc.vector.tensor_add(o[:], t1[:], b2_sb[:])
                nc.sync.dma_start(out_d[i * P:(i + 1) * P, :], o[:])

            if "C" in phases:
                if "D" in phases:
                    aggregate(x2full, "N", consume2)
                else:
                    for i in range(BPC):
                        o = eppool.tile([P, F], f32, tag="xo")
                        nc.sync.dma_start(o[:], x2full[i * P:(i + 1) * P, :])
                        nc.sync.dma_start(out_d[i * P:(i + 1) * P, :], o[:])

    nc.compile()
    return nc


def make_in_maps(plan, table_dtype="float32"):
    import ml_dtypes

    cast = {
        "float32": np.float32,
        "bfloat16": ml_dtypes.bfloat16,
    }[table_dtype]
    maps = []
    for c in range(plan.NC):
        maps.append(
            {
                "zT": plan.zT[c],
                "W1": plan.W1,
                "W2": plan.W2,
                "b1c": plan.b1c,
                "b2b": plan.b2b,
                "iota": plan.iota.astype(cast),
                "iotar": plan.iotar.astype(cast),
                "dinvc": plan.dinvc[c],
                "dinvb": plan.dinvb[c],
                "gidx": plan.gidx[c],
                "dloc": plan.dloc[c].astype(cast),
            }
        )
    return maps


_CACHE = {}


def _run(z, ei, W1, b1, W2, b2, n_cores=NC_DEFAULT, table_dtype="bfloat16",
         trace=False):
    from concourse.bass_utils import run_bass_kernel_spmd

    plan = GCNPlan(z, ei, W1, b1, W2, b2, n_cores=n_cores)
    key = (plan.N, plan.TCH, table_dtype)
    if key not in _CACHE:
        _CACHE[key] = build_bass(plan, table_dtype=table_dtype,
                                 n_queues=4, gather_split=1024)
    nc = _CACHE[key]
    in_maps = make_in_maps(plan, table_dtype=table_dtype)
    res = run_bass_kernel_spmd(
        nc, in_maps, list(range(n_cores)), trace=trace
    )
    outp = np.concatenate([res.results[c]["out"] for c in range(plan.NC)], axis=0)
    out = outp[plan.perm[: plan.N]].astype(np.float32)
    return out, res


def kernel(z, ei, W1, b1, W2, b2):
    out, _ = _run(
        np.asarray(z), np.asarray(ei), np.asarray(W1), np.asarray(b1),
        np.asarray(W2), np.asarray(b2), table_dtype="bfloat16",
    )
    return out
